# revision 85
# baseline (speedup 1.0000x reference)
"""Trainium2 Bass kernel for nn_EnrichBlock (B=2,S=2048,D=512,H=8,FF=2048).

Sharding: token-parallel over (batch, seq) -> 8 shards of 512 query tokens.
No collectives: each core recomputes K/V projections for its batch element
from the raw inputs (all cross-token deps flow through raw x_q/x1/x2).

Host prep: weights pre-transposed to [in,out] (Q-side pre-scaled by 1/8),
activations pre-transposed to feature-major [D, T], per-core causal mask in
key-major layout. All device DMAs are then contiguous.

On-chip: bf16 matmul operands, fp32 PSUM/LN/softmax-denominators. Attention
is computed key-major (scoresT[k,q] = K_h^T-chunk x Q_h), causal mask is
PSUM-injected via an identity matmul, exp on ACT, softmax denominator via an
appended ones-column in V, then a per-head PE transpose back to token-major
with the denominator riding along as a 65th row; normalization, head-merge,
residual and LayerNorm all happen token-major.
"""

import numpy as np
import ml_dtypes

import concourse.bass as bass
import concourse.mybir as mybir
import concourse.tile as tile
from concourse.bacc import Bacc
from concourse.masks import make_identity
from concourse.tile_rust import add_dep_helper
from concourse.bass_utils import run_bass_kernel_spmd

B, S, D, H, FF = 2, 2048, 512, 8, 2048
DK = D // H          # 64
T = 512              # query tokens per core
NC = 8               # cores
EPS = 1e-5
P = 128
F32 = mybir.dt.float32
BF16 = mybir.dt.bfloat16
AF = mybir.ActivationFunctionType
ALU = mybir.AluOpType

KC = D // P          # 4   contraction chunks over D
QC = T // P          # 4   query-token chunks
TC = S // P          # 16  key-token chunks
FFC = FF // P        # 16


def build_nc():
    nc = Bacc(num_devices=NC)

    def dma(engine, dst, src):
        return engine.dma_start(out=dst, in_=src)

    # ---- DRAM tensors ----
    di = lambda n, sh, dt: nc.dram_tensor(n, sh, dt, kind="ExternalInput")
    xqT_own = di("xqT_own", [D, T], BF16)       # feature-major own slice
    xqT_full = di("xqT_full", [D, S], BF16)     # feature-major full batch elem
    x1T_own = di("x1T_own", [D, T], BF16)       # own token slice only (TP)
    x2T_own = di("x2T_own", [D, T], BF16)
    # staging for the 4-way K/V AllGathers (tensor-parallel projections)
    VW = H * (DK + 1)
    st = {}
    for nm in ["k1", "v1", "k2", "v2"]:
        inner = [D, T] if nm[0] == "k" else [T, VW]
        st[nm + "i"] = nc.dram_tensor(nm + "i", inner, BF16, kind="Internal")
        st[nm + "o"] = nc.dram_tensor(nm + "o", [4] + inner, BF16,
                                      kind="Internal")
    GROUPS = [[0, 1, 2, 3], [4, 5, 6, 7]]
    xq_own_tm = di("xq_own_tm", [T, D], F32)    # token-major own slice
    maskT = di("maskT", [S, T], BF16)           # additive causal mask, key-major

    w_saqT = di("w_saqT", [D, D], BF16)         # sa_wq.T / 8
    w_sakT = di("w_sakT", [D, D], BF16)
    w_savT = di("w_savT", [D, D], BF16)
    w_mqT = di("w_mqT", [D, D], BF16)           # mha Wq.T / 8
    w_mkT = di("w_mkT", [D, D], BF16)
    w_mvT = di("w_mvT", [D, D], BF16)
    w_moT = di("w_moT", [D, D], BF16)
    w_f1T = di("w_f1T", [D, FF], BF16)
    w_f2T = di("w_f2T", [FF, D], BF16)

    b_saq = di("b_saq", [P, KC], F32)           # packed per-partition biases
    b_sak = di("b_sak", [P, KC], F32)
    b_mq = di("b_mq", [P, KC], F32)
    b_mk = di("b_mk", [P, KC], F32)
    b_f1 = di("b_f1", [P, FFC], F32)
    b_l1g = di("b_l1g", [P, KC], F32)
    b_l1b = di("b_l1b", [P, KC], F32)
    b_l2g = di("b_l2g", [P, KC], F32)
    b_l2b = di("b_l2b", [P, KC], F32)

    # row vectors, broadcast over partitions at load time
    rows = {}
    for n in ["bv_sa", "bv_m", "bmo", "bf2",
              "ln1g", "ln1b", "ln2g", "ln2b", "ln3g", "ln3b"]:
        rows[n] = di(n, [D], F32)

    out = nc.dram_tensor("out", [T, D], F32, kind="ExternalOutput")

    with tile.TileContext(nc) as tc:
        with (
            tc.tile_pool(name="singles", bufs=1) as singles,
            tc.tile_pool(name="xpool", bufs=2) as xpool,      # 2 x 2MB slots
            tc.tile_pool(name="kpool", bufs=2) as kpool,      # kT / w2T slots
            tc.tile_pool(name="vpool", bufs=1) as vpool,
            tc.tile_pool(name="mpool", bufs=1) as mpool,      # maskT / hT slot
            tc.tile_pool(name="qpool", bufs=1) as qpool,
            tc.tile_pool(name="epool", bufs=7) as epool,
            tc.tile_pool(name="apool", bufs=2) as apool,
            tc.tile_pool(name="tmpool", bufs=2) as tmpool,
            tc.tile_pool(name="rpool", bufs=2) as rpool,
            tc.tile_pool(name="tppool", bufs=1) as tppool,
            tc.tile_pool(name="fmpool", bufs=1) as fmpool,
            tc.tile_pool(name="bcpool", bufs=2) as bcpool,
            tc.tile_pool(name="stat", bufs=6) as stat,
            tc.tile_pool(name="psA", bufs=2, space="PSUM") as psA,
            tc.tile_pool(name="psB", bufs=2, space="PSUM") as psB,
            tc.tile_pool(name="psC", bufs=2, space="PSUM") as psC,
        ):
            # ---- constants / persistent loads ----
            id_f32 = singles.tile([P, P], F32, tag="id_f32")
            make_identity(nc, id_f32)
            eps_sb = singles.tile([P, 1], F32, tag="eps")
            nc.vector.memset(eps_sb, EPS)

            def load_w(name, t, icnk, ocols):  # [I,O] dram -> [128, icnk, ocols]
                w = singles.tile([P, icnk, ocols], BF16, tag=name)
                dma(nc.sync, w, t.rearrange("(c p) o -> p c o", p=P))
                return w

            def load_b(name, t, cols):
                b = singles.tile([P, cols], F32, tag=name)
                dma(nc.sync, b, t[:, :])
                return b

            def bcast_row(handle):  # [D] dram row -> [128, D] sbuf
                t = bcpool.tile([P, D], F32, tag="bc")
                src = bass.AP(
                    tensor=handle[:].tensor,
                    offset=handle[:].offset,
                    ap=[[0, P], [1, D]],
                )
                dma(nc.gpsimd, t, src)
                return t

            def load_xT(handle):  # [D, S] -> [128, KC, S]
                t = xpool.tile([P, KC, S], BF16, tag="xslot")
                dma(nc.sync, t, handle.rearrange("(c p) t -> p c t", p=P))
                return t

            # loads ordered so SA's first matmuls can start early
            saq = load_w("saq", w_saqT, KC, D)
            bsaq = load_b("bsaq", b_saq, KC)
            xqo_sb = singles.tile([P, KC, T], BF16, tag="xqo")
            dma(nc.sync, xqo_sb, xqT_own.rearrange("(c p) t -> p c t", p=P))
            sak = load_w("sak", w_sakT, KC, D)
            bsak = load_b("bsak", b_sak, KC)
            xq_sb = load_xT(xqT_full)
            sav = load_w("sav", w_savT, KC, D)
            mask_sb = mpool.tile([P, TC, T], BF16, tag="mslot")
            dma(nc.sync, mask_sb, maskT.rearrange("(c p) q -> p c q", p=P))
            xq_tm = tmpool.tile([P, QC, D], F32, tag="tmslot")
            dma(nc.sync, xq_tm, xq_own_tm.rearrange("(c p) d -> p c d", p=P))
            mq = load_w("mq", w_mqT, KC, D)
            mk = load_w("mk", w_mkT, KC, D)
            mv = load_w("mv", w_mvT, KC, D)
            mo = load_w("mo", w_moT, KC, D)
            bmq = load_b("bmq", b_mq, KC)
            bmk = load_b("bmk", b_mk, KC)
            bf1 = load_b("bf1", b_f1, FFC)
            l1g = load_b("l1g", b_l1g, KC)
            l1b = load_b("l1b", b_l1b, KC)
            l2g = load_b("l2g", b_l2g, KC)
            l2b = load_b("l2b", b_l2b, KC)
            x1o_sb = singles.tile([P, KC, T], BF16, tag="x1o")
            dma(nc.sync, x1o_sb, x1T_own.rearrange("(c p) t -> p c t", p=P))
            x2o_sb = singles.tile([P, KC, T], BF16, tag="x2o")
            dma(nc.sync, x2o_sb, x2T_own.rearrange("(c p) t -> p c t", p=P))

            # ---- helpers ----
            def proj_fm(wt, xt, bias, ntok, name):
                """OT fm [128, KC, ntok] bf16 = wt.T-style proj of xt + bias."""
                dst = qpool.tile([P, KC, ntok], BF16, tag=f"prj_{ntok}")
                for oc in range(KC):
                    for nk in range(ntok // 512):
                        ps = psA.tile([P, 512], F32, tag="big")
                        for kc in range(KC):
                            nc.tensor.matmul(
                                ps,
                                wt[:, kc, oc * P:(oc + 1) * P],
                                xt[:, kc, nk * 512:(nk + 1) * 512],
                                start=(kc == 0),
                                stop=(kc == KC - 1),
                            )
                        if bias is not None:
                            nc.vector.tensor_scalar_add(
                                dst[:, oc, nk * 512:(nk + 1) * 512],
                                ps, bias[:, oc:oc + 1])
                        else:
                            nc.vector.tensor_copy(
                                dst[:, oc, nk * 512:(nk + 1) * 512], ps)
                return dst

            def proj_v_tm(wt, xt):
                """V token-major with ones column: [128, TC, 8*65] bf16."""
                v = vpool.tile([P, TC, H * (DK + 1)], BF16, tag="vslot")
                v4 = v.rearrange("p t (h w) -> p t h w", w=DK + 1)
                nc.vector.memset(v4[:, :, :, DK:DK + 1], 1.0)
                for tcx in range(TC):
                    ps = psA.tile([P, 512], F32, tag="big")
                    for kc in range(KC):
                        nc.tensor.matmul(
                            ps,
                            xt[:, kc, tcx * P:(tcx + 1) * P],
                            wt[:, kc, :],
                            start=(kc == 0),
                            stop=(kc == KC - 1),
                        )
                    nc.vector.tensor_copy(
                        v4[:, tcx, :, 0:DK],
                        ps.rearrange("p (h w) -> p h w", w=DK),
                    )
                return v

            def tp_kv_out(xo_sb, ki, ko, vi, vo):
                """Project own-token K/V slices, stage to DRAM, AllGather
                across the 4-core batch group."""
                k_own = tppool.tile([P, KC, T], BF16, tag="tp", name="ko")
                for oc in range(KC):
                    ps = psC.tile([P, 512], F32, tag="tr")
                    for kc in range(KC):
                        nc.tensor.matmul(
                            ps, mk[:, kc, oc * P:(oc + 1) * P],
                            xo_sb[:, kc, :],
                            start=(kc == 0), stop=(kc == KC - 1))
                    nc.vector.tensor_scalar_add(k_own[:, oc, :], ps,
                                                bmk[:, oc:oc + 1])
                dk = dma(nc.sync, ki.rearrange("(c p) t -> p c t", p=P), k_own)
                ck = nc.gpsimd.collective_compute(
                    kind="AllGather", op=ALU.bypass, replica_groups=GROUPS,
                    ins=[ki[:]], outs=[ko[:]])
                add_dep_helper(ck.ins, dk.ins, sync=True, reason="cc in")
                v_own = tppool.tile([P, QC, VW], BF16, tag="tp", name="vo")
                v4 = v_own.rearrange("p t (h w) -> p t h w", w=DK + 1)
                nc.vector.memset(v4[:, :, :, DK:DK + 1], 1.0)
                for tcl in range(QC):
                    ps = psC.tile([P, 512], F32, tag="tr")
                    for kc in range(KC):
                        nc.tensor.matmul(
                            ps, xo_sb[:, kc, tcl * P:(tcl + 1) * P],
                            mv[:, kc, :],
                            start=(kc == 0), stop=(kc == KC - 1))
                    nc.vector.tensor_copy(
                        v4[:, tcl, :, 0:DK],
                        ps.rearrange("p (h w) -> p h w", w=DK))
                dv = dma(nc.sync, vi.rearrange("(c p) w -> p c w", p=P), v_own)
                cv = nc.gpsimd.collective_compute(
                    kind="AllGather", op=ALU.bypass, replica_groups=GROUPS,
                    ins=[vi[:]], outs=[vo[:]])
                add_dep_helper(cv.ins, dv.ins, sync=True, reason="cc in")
                return ck, cv

            def tp_kv_in(ko, vo, ck, cv):
                kT_full = kpool.tile([P, KC, S], BF16, tag="kslot",
                                     name="ktf")
                v_full = vpool.tile([P, TC, VW], BF16, tag="vslot",
                                    name="vtf")
                for m in range(4):
                    dk = dma(nc.sync, kT_full[:, :, m * T:(m + 1) * T],
                             ko[m].rearrange("(c p) t -> p c t", p=P))
                    add_dep_helper(dk.ins, ck.ins, sync=True, reason="cc out")
                    dv = dma(nc.sync, v_full[:, m * QC:(m + 1) * QC, :],
                             vo[m].rearrange("(c p) w -> p c w", p=P))
                    add_dep_helper(dv.ins, cv.ins, sync=True, reason="cc out")
                return kT_full, v_full

            def attention(qT, kT, v, masked, bg=None, tail_bg=None):
                """-> attn_tm [128, QC, D] f32, normalized (no v-bias yet).

                Head-pair loop: the two half-array (K=64) score matmuls of a
                pair target row groups 0:64 / 64:128 and disjoint bank halves
                of one 2-bank PSUM tile, so they run concurrently; one wide
                EXP covers both heads."""
                attn = tmpool.tile([P, QC, D], F32, tag="tmslot")
                for j in range(H // 2):
                    h0, h1 = 2 * j, 2 * j + 1
                    pa0 = psB.tile([DK + 1, 512], F32, tag="av")
                    pa1 = psB.tile([DK + 1, 512], F32, tag="av")
                    ets = {}
                    LAG = 3
                    for tcx in range(TC + LAG):
                        if tcx < TC:
                            ps = psA.tile([P, 1024], F32, tag="big")
                            nc.tensor.matmul(
                                ps[:, 0:512],
                                kT[0:DK, j, tcx * P:(tcx + 1) * P],
                                qT[0:DK, j, :], start=True, stop=True)
                            nc.tensor.matmul(
                                ps[:, 512:1024],
                                kT[DK:2 * DK, j, tcx * P:(tcx + 1) * P],
                                qT[DK:2 * DK, j, :], start=True, stop=True)
                            et = epool.tile([P, 1024], BF16, tag="exp")
                            nc.scalar.activation(et, ps, AF.Exp)
                            if masked:
                                nc.vector.tensor_tensor(
                                    et[:, 0:512], et[:, 0:512],
                                    mask_sb[:, tcx, :], ALU.mult)
                                nc.vector.tensor_tensor(
                                    et[:, 512:1024], et[:, 512:1024],
                                    mask_sb[:, tcx, :], ALU.mult)
                            ets[tcx] = et
                        if tcx >= LAG:
                            t0 = tcx - LAG
                            et = ets.pop(t0)
                            nc.tensor.matmul(
                                pa0,
                                v[:, t0, h0 * (DK + 1):(h0 + 1) * (DK + 1)],
                                et[:, 0:512],
                                start=(t0 == 0), stop=(t0 == TC - 1))
                            nc.tensor.matmul(
                                pa1,
                                v[:, t0, h1 * (DK + 1):(h1 + 1) * (DK + 1)],
                                et[:, 512:1024],
                                start=(t0 == 0), stop=(t0 == TC - 1))
                        if bg and (tcx % 2 == 0 or len(bg) > 24):
                            bg.pop(0)()
                    for h, pa in ((h0, pa0), (h1, pa1)):
                        av = apool.tile([DK + 1, 512], F32, tag="avsb")
                        nc.vector.tensor_copy(av, pa)
                        for qcx in range(QC):
                            pt = psC.tile([P, DK + 1], F32, tag="tr")
                            nc.tensor.transpose(
                                pt, av[:, qcx * P:(qcx + 1) * P],
                                id_f32[:DK + 1, :DK + 1])
                            rc = stat.tile([P, 1], F32, tag="rc")
                            nc.vector.reciprocal(rc, pt[:, DK:DK + 1])
                            nc.vector.tensor_scalar_mul(
                                attn[:, qcx, h * DK:(h + 1) * DK],
                                pt[:, 0:DK], rc)
                while bg:
                    bg.pop(0)()
                # units that write tiles whose slot is released only by this
                # attention's final AV matmuls (emitting them earlier wedges
                # the engine queues behind the WAR wait)
                while tail_bg:
                    tail_bg.pop(0)()
                return attn

            def ln_stats(mvall, z, qcx):
                st = stat.tile([P, nc.vector.BN_STATS_DIM], F32, tag="st")
                nc.vector.bn_stats(st, z[:, qcx, :])
                nc.vector.bn_aggr(mvall[:, qcx, :], st)

            def ln_apply(mvall, z, g_row, b_row, gb_pk=None, out_ap=None):
                """Normalize z in place given precomputed stats. If gb_pk is
                given, also emit the fm transpose per query-chunk: transposes
                read the pre-affine normalized z (no wait on the tm affine),
                and g/b are applied per-partition on the psum->fm copy."""
                g_bc = bcast_row(g_row)
                b_bc = bcast_row(b_row)
                if gb_pk is not None:
                    dst = fmpool.tile([P, KC, T], BF16, tag="fm", name="fmt")
                    g_pk, b_pk = gb_pk
                else:
                    dst = None
                for qcx in range(QC):
                    sd = stat.tile([P, 1], F32, tag="sds")
                    nc.scalar.activation(sd, mvall[:, qcx, 1:2], AF.Sqrt,
                                         bias=eps_sb, scale=1.0)
                    rstd = stat.tile([P, 1], F32, tag="rstds")
                    nc.vector.reciprocal(rstd, sd)
                    nc.vector.tensor_scalar(
                        z[:, qcx, :], z[:, qcx, :],
                        mvall[:, qcx, 0:1], rstd,
                        op0=ALU.subtract, op1=ALU.mult)
                    if dst is not None:
                        for dc in range(KC):
                            pt = psC.tile([P, P], F32, tag="tr")
                            nc.tensor.transpose(
                                pt, z[:, qcx, dc * P:(dc + 1) * P], id_f32)
                            nc.vector.tensor_scalar(
                                dst[:, dc, qcx * P:(qcx + 1) * P], pt,
                                g_pk[:, dc:dc + 1], b_pk[:, dc:dc + 1],
                                op0=ALU.mult, op1=ALU.add)
                    nc.vector.tensor_tensor(z[:, qcx, :], z[:, qcx, :], g_bc,
                                            ALU.mult)
                    nc.vector.tensor_add(z[:, qcx, :], z[:, qcx, :], b_bc)
                    if out_ap is not None:
                        dma(nc.sync, out_ap[:, qcx, :], z[:, qcx, :])
                return dst

            def transpose_tm_to_fm(src_tm):
                dst = fmpool.tile([P, KC, T], BF16, tag="fm")
                for qcx in range(QC):
                    for dc in range(KC):
                        pt = psC.tile([P, P], F32, tag="tr")
                        nc.tensor.transpose(
                            pt, src_tm[:, qcx, dc * P:(dc + 1) * P], id_f32)
                        nc.vector.tensor_copy(
                            dst[:, dc, qcx * P:(qcx + 1) * P], pt)
                return dst

            # ======== self-attention ========
            qT = proj_fm(saq, xqo_sb, bsaq, T, "saQ")
            kT = kpool.tile([P, KC, S], BF16, tag="kslot")
            for oc in range(KC):
                for nk in range(S // 512):
                    ps = psA.tile([P, 512], F32, tag="big")
                    for kc in range(KC):
                        nc.tensor.matmul(
                            ps, sak[:, kc, oc * P:(oc + 1) * P],
                            xq_sb[:, kc, nk * 512:(nk + 1) * 512],
                            start=(kc == 0), stop=(kc == KC - 1))
                    nc.vector.tensor_scalar_add(
                        kT[:, oc, nk * 512:(nk + 1) * 512], ps,
                        bsak[:, oc:oc + 1])
            v = proj_v_tm(sav, xq_sb)

            # TP: project own K/V slices for both cross-attns, AllGather
            ck1, cv1 = tp_kv_out(x1o_sb, st["k1i"], st["k1o"],
                                 st["v1i"], st["v1o"])
            kT_ca1, v_ca1 = tp_kv_in(st["k1o"], st["v1o"], ck1, cv1)

            attn = attention(qT, kT, v, masked=True)
            # second TP projection emitted here: its 32 matmuls fill the
            # PE gap while the SA->CA1 LayerNorm chain runs on DVE
            ck2, cv2 = tp_kv_out(x2o_sb, st["k2i"], st["k2o"],
                                 st["v2i"], st["v2o"])
            bv_bc = bcast_row(rows["bv_sa"])
            z1 = rpool.tile([P, QC, D], F32, tag="resid")
            mv1 = stat.tile([P, QC, 2], F32, tag="mvall")
            for qcx in range(QC):
                nc.vector.tensor_add(attn[:, qcx, :], attn[:, qcx, :], bv_bc)
                nc.vector.tensor_tensor(z1[:, qcx, :], xq_tm[:, qcx, :],
                                        attn[:, qcx, :], ALU.add)
                ln_stats(mv1, z1, qcx)
            yT1 = ln_apply(mv1, z1, rows["ln1g"], rows["ln1b"],
                           gb_pk=(l1g, l1b))

            # ======== cross-attention block (shared weights), used twice ====
            def cross_block(y_tm, yT, kTc, vc, bg, tail_bg, last=False):
                qTc = proj_fm(mq, yT, bmq, T, "q")
                attnc = attention(qTc, kTc, vc, masked=False, bg=bg,
                                  tail_bg=tail_bg)
                bvm_bc = bcast_row(rows["bv_m"])
                for qcx in range(QC):
                    nc.vector.tensor_add(attnc[:, qcx, :], attnc[:, qcx, :],
                                         bvm_bc)
                attnT = transpose_tm_to_fm(attnc)
                # out-projection directly token-major:
                # psum[tok, outD] = sum_kc attnT[:,kc,tok-chunk].T @ moT[:,kc,:]
                bmo_bc = bcast_row(rows["bmo"])
                z = rpool.tile([P, QC, D], F32, tag="resid")
                mvc = stat.tile([P, QC, 2], F32, tag="mvall")
                for qcx in range(QC):
                    ps = psA.tile([P, 512], F32, tag="big")
                    for kc in range(KC):
                        nc.tensor.matmul(
                            ps, attnT[:, kc, qcx * P:(qcx + 1) * P],
                            mo[:, kc, :],
                            start=(kc == 0), stop=(kc == KC - 1))
                    nc.vector.tensor_tensor(z[:, qcx, :], ps, y_tm[:, qcx, :],
                                            ALU.add)
                    nc.vector.tensor_add(z[:, qcx, :], z[:, qcx, :], bmo_bc)
                    ln_stats(mvc, z, qcx)
                zT = ln_apply(mvc, z, rows["ln2g"], rows["ln2b"],
                              gb_pk=(l2g, l2b))
                return z, zT

            kT_ca2, v_ca2 = tp_kv_in(st["k2o"], st["v2o"], ck2, cv2)
            y1, yT2 = cross_block(z1, yT1, kT_ca1, v_ca1, None, None)
            y2, y2T = cross_block(y1, yT2, kT_ca2, v_ca2, None, None)

            # ======== FFN ========
            w1 = xpool.tile([P, KC, FF], BF16, tag="xslot")
            dma(nc.sync, w1, w_f1T.rearrange("(c p) o -> p c o", p=P))
            w2 = kpool.tile([P, FFC, D], BF16, tag="kslot")
            dma(nc.sync, w2, w_f2T.rearrange("(c p) o -> p c o", p=P))

            hT = mpool.tile([P, FFC, T], BF16, tag="mslot")
            for oc in range(FFC):
                ps = psA.tile([P, 512], F32, tag="big")
                for kc in range(KC):
                    nc.tensor.matmul(
                        ps, w1[:, kc, oc * P:(oc + 1) * P], y2T[:, kc, :],
                        start=(kc == 0), stop=(kc == KC - 1))
                nc.scalar.activation(hT[:, oc, :], ps, AF.Relu,
                                     bias=bf1[:, oc:oc + 1], scale=1.0)
            bf2_bc = bcast_row(rows["bf2"])
            z3 = rpool.tile([P, QC, D], F32, tag="resid")
            mv3 = stat.tile([P, QC, 2], F32, tag="mvall")
            for qcx in range(QC):
                ps = psA.tile([P, 512], F32, tag="big")
                for kc in range(FFC):
                    nc.tensor.matmul(
                        ps, hT[:, kc, qcx * P:(qcx + 1) * P], w2[:, kc, :],
                        start=(kc == 0), stop=(kc == FFC - 1))
                nc.vector.tensor_tensor(z3[:, qcx, :], ps, y2[:, qcx, :],
                                        ALU.add)
                nc.vector.tensor_add(z3[:, qcx, :], z3[:, qcx, :], bf2_bc)
                ln_stats(mv3, z3, qcx)
            ln_apply(mv3, z3, rows["ln3g"], rows["ln3b"],
                     out_ap=out.rearrange("(c p) d -> p c d", p=P))

    nc.finalize()
    return nc


_NC_CACHE = None


def _prep_inputs(x_q, x1, x2, sa_wq, sa_bq, sa_wk, sa_bk, sa_wv, sa_bv,
                 ln1_g, ln1_b, mha_in_w, mha_in_b, mha_out_w, mha_out_b,
                 ln2_g, ln2_b, ffn_w1, ffn_b1, ffn_w2, ffn_b2, ln3_g, ln3_b):
    bf = ml_dtypes.bfloat16
    f32 = np.float32
    scale = 1.0 / np.sqrt(np.float32(DK))

    def pk(b):  # [O] -> [128, O//128] per-partition packed
        return np.ascontiguousarray(np.asarray(b, f32).reshape(-1, P).T)

    shared = {
        "w_saqT": np.ascontiguousarray((sa_wq * scale).T.astype(bf)),
        "w_sakT": np.ascontiguousarray(sa_wk.T.astype(bf)),
        "w_savT": np.ascontiguousarray(sa_wv.T.astype(bf)),
        "w_mqT": np.ascontiguousarray((mha_in_w[:D] * scale).T.astype(bf)),
        "w_mkT": np.ascontiguousarray(mha_in_w[D:2 * D].T.astype(bf)),
        "w_mvT": np.ascontiguousarray(mha_in_w[2 * D:].T.astype(bf)),
        "w_moT": np.ascontiguousarray(mha_out_w.T.astype(bf)),
        "w_f1T": np.ascontiguousarray(ffn_w1.T.astype(bf)),
        "w_f2T": np.ascontiguousarray(ffn_w2.T.astype(bf)),
        "b_saq": pk(sa_bq * scale),
        "b_sak": pk(sa_bk),
        "b_mq": pk(mha_in_b[:D] * scale),
        "b_mk": pk(mha_in_b[D:2 * D]),
        "b_f1": pk(ffn_b1),
        "b_l1g": pk(ln1_g), "b_l1b": pk(ln1_b),
        "b_l2g": pk(ln2_g), "b_l2b": pk(ln2_b),
        "bv_sa": np.asarray(sa_bv, f32),
        "bv_m": np.asarray(mha_in_b[2 * D:], f32),
        "bmo": np.asarray(mha_out_b, f32),
        "bf2": np.asarray(ffn_b2, f32),
        "ln1g": np.asarray(ln1_g, f32), "ln1b": np.asarray(ln1_b, f32),
        "ln2g": np.asarray(ln2_g, f32), "ln2b": np.asarray(ln2_b, f32),
        "ln3g": np.asarray(ln3_g, f32), "ln3b": np.asarray(ln3_b, f32),
    }

    kk = np.arange(S, dtype=np.int64)[:, None]
    in_maps = []
    for c in range(NC):
        b, s = c // 4, c % 4
        xT = np.ascontiguousarray(x_q[b].T.astype(bf))      # [D, S]
        qq = np.arange(T, dtype=np.int64)[None, :] + s * T
        m = np.where(kk <= qq, 1.0, 0.0).astype(bf)         # [S, T] 0/1 mult mask
        im = dict(shared)
        im["xqT_full"] = xT
        im["xqT_own"] = np.ascontiguousarray(xT[:, s * T:(s + 1) * T])
        im["x1T_own"] = np.ascontiguousarray(
            x1[b].T[:, s * T:(s + 1) * T].astype(bf))
        im["x2T_own"] = np.ascontiguousarray(
            x2[b].T[:, s * T:(s + 1) * T].astype(bf))
        im["xq_own_tm"] = np.ascontiguousarray(
            x_q[b, s * T:(s + 1) * T, :].astype(f32))
        im["maskT"] = m
        in_maps.append(im)
    return in_maps


def kernel(**inputs):
    global _NC_CACHE
    if _NC_CACHE is None:
        _NC_CACHE = build_nc()
    nc = _NC_CACHE
    in_maps = _prep_inputs(**{k: np.asarray(v) for k, v in inputs.items()})
    res = run_bass_kernel_spmd(nc, in_maps, core_ids=list(range(NC)))
    full = np.empty((B, S, D), np.float32)
    for c in range(NC):
        b, s = c // 4, c % 4
        full[b, s * T:(s + 1) * T, :] = res.results[c]["out"]
    return full


# revision 88
# speedup vs baseline: 1.0149x; 1.0149x over previous
"""Trainium2 Bass kernel for nn_EnrichBlock (B=2,S=2048,D=512,H=8,FF=2048).

Sharding: token-parallel over (batch, seq) -> 8 shards of 512 query tokens.
No collectives: each core recomputes K/V projections for its batch element
from the raw inputs (all cross-token deps flow through raw x_q/x1/x2).

Host prep: weights pre-transposed to [in,out] (Q-side pre-scaled by 1/8),
activations pre-transposed to feature-major [D, T], per-core causal mask in
key-major layout. All device DMAs are then contiguous.

On-chip: bf16 matmul operands, fp32 PSUM/LN/softmax-denominators. Attention
is computed key-major (scoresT[k,q] = K_h^T-chunk x Q_h), causal mask is
PSUM-injected via an identity matmul, exp on ACT, softmax denominator via an
appended ones-column in V, then a per-head PE transpose back to token-major
with the denominator riding along as a 65th row; normalization, head-merge,
residual and LayerNorm all happen token-major.
"""

import numpy as np
import ml_dtypes

import concourse.bass as bass
import concourse.mybir as mybir
import concourse.tile as tile
from concourse.bacc import Bacc
from concourse.masks import make_identity
from concourse.tile_rust import add_dep_helper
from concourse.bass_utils import run_bass_kernel_spmd

B, S, D, H, FF = 2, 2048, 512, 8, 2048
DK = D // H          # 64
T = 512              # query tokens per core
NC = 8               # cores
EPS = 1e-5
P = 128
F32 = mybir.dt.float32
BF16 = mybir.dt.bfloat16
AF = mybir.ActivationFunctionType
ALU = mybir.AluOpType

KC = D // P          # 4   contraction chunks over D
QC = T // P          # 4   query-token chunks
TC = S // P          # 16  key-token chunks
FFC = FF // P        # 16


def build_nc():
    nc = Bacc(num_devices=NC)

    def dma(engine, dst, src):
        return engine.dma_start(out=dst, in_=src)

    # ---- DRAM tensors ----
    di = lambda n, sh, dt: nc.dram_tensor(n, sh, dt, kind="ExternalInput")
    xqT_own = di("xqT_own", [D, T], BF16)       # feature-major own slice
    xqT_full = di("xqT_full", [D, S], BF16)     # feature-major full batch elem
    x1T_own = di("x1T_own", [D, T], BF16)       # own token slice only (TP)
    x2T_own = di("x2T_own", [D, T], BF16)
    # staging for the 4-way K/V AllGathers (tensor-parallel projections)
    VW = H * (DK + 1)
    st = {}
    for nm in ["k1", "v1", "k2", "v2"]:
        inner = [D, T] if nm[0] == "k" else [T, VW]
        st[nm + "i"] = nc.dram_tensor(nm + "i", inner, BF16, kind="Internal")
        st[nm + "o"] = nc.dram_tensor(nm + "o", [4] + inner, BF16,
                                      kind="Internal")
    GROUPS = [[0, 1, 2, 3], [4, 5, 6, 7]]
    xq_own_tm = di("xq_own_tm", [T, D], F32)    # token-major own slice
    maskT = di("maskT", [S, T], BF16)           # additive causal mask, key-major

    w_saqT = di("w_saqT", [D, D], BF16)         # sa_wq.T / 8
    w_sakT = di("w_sakT", [D, D], BF16)
    w_savT = di("w_savT", [D, D], BF16)
    w_mqT = di("w_mqT", [D, D], BF16)           # mha Wq.T / 8
    w_mkT = di("w_mkT", [D, D], BF16)
    w_mvT = di("w_mvT", [D, D], BF16)
    w_moT = di("w_moT", [D, D], BF16)
    w_f1T = di("w_f1T", [D, FF], BF16)
    w_f2T = di("w_f2T", [FF, D], BF16)

    b_saq = di("b_saq", [P, KC], F32)           # packed per-partition biases
    b_sak = di("b_sak", [P, KC], F32)
    b_mq = di("b_mq", [P, KC], F32)
    b_mk = di("b_mk", [P, KC], F32)
    b_f1 = di("b_f1", [P, FFC], F32)
    b_l1g = di("b_l1g", [P, KC], F32)
    b_l1b = di("b_l1b", [P, KC], F32)
    b_l2g = di("b_l2g", [P, KC], F32)
    b_l2b = di("b_l2b", [P, KC], F32)

    # row vectors, broadcast over partitions at load time
    rows = {}
    for n in ["bv_sa", "bv_m", "bmo", "bf2",
              "ln1g", "ln1b", "ln2g", "ln2b", "ln3g", "ln3b"]:
        rows[n] = di(n, [D], F32)

    out = nc.dram_tensor("out", [T, D], F32, kind="ExternalOutput")

    with tile.TileContext(nc) as tc:
        with (
            tc.tile_pool(name="singles", bufs=1) as singles,
            tc.tile_pool(name="xpool", bufs=2) as xpool,      # 2 x 2MB slots
            tc.tile_pool(name="kpool", bufs=2) as kpool,      # kT / w2T slots
            tc.tile_pool(name="vpool", bufs=1) as vpool,
            tc.tile_pool(name="mpool", bufs=1) as mpool,      # maskT / hT slot
            tc.tile_pool(name="qpool", bufs=1) as qpool,
            tc.tile_pool(name="epool", bufs=6) as epool,
            tc.tile_pool(name="apool", bufs=2) as apool,
            tc.tile_pool(name="tmpool", bufs=2) as tmpool,
            tc.tile_pool(name="rpool", bufs=2) as rpool,
            tc.tile_pool(name="tppool", bufs=1) as tppool,
            tc.tile_pool(name="fmpool", bufs=1) as fmpool,
            tc.tile_pool(name="bcpool", bufs=2) as bcpool,
            tc.tile_pool(name="stat", bufs=6) as stat,
            tc.tile_pool(name="psA", bufs=2, space="PSUM") as psA,
            tc.tile_pool(name="psB", bufs=2, space="PSUM") as psB,
            tc.tile_pool(name="psC", bufs=2, space="PSUM") as psC,
        ):
            # ---- constants / persistent loads ----
            id_f32 = singles.tile([P, P], F32, tag="id_f32")
            make_identity(nc, id_f32)
            eps_sb = singles.tile([P, 1], F32, tag="eps")
            nc.vector.memset(eps_sb, EPS)

            def load_w(name, t, icnk, ocols):  # [I,O] dram -> [128, icnk, ocols]
                w = singles.tile([P, icnk, ocols], BF16, tag=name)
                dma(nc.sync, w, t.rearrange("(c p) o -> p c o", p=P))
                return w

            def load_b(name, t, cols):
                b = singles.tile([P, cols], F32, tag=name)
                dma(nc.sync, b, t[:, :])
                return b

            def bcast_row(handle):  # [D] dram row -> [128, D] sbuf
                t = bcpool.tile([P, D], F32, tag="bc")
                src = bass.AP(
                    tensor=handle[:].tensor,
                    offset=handle[:].offset,
                    ap=[[0, P], [1, D]],
                )
                dma(nc.gpsimd, t, src)
                return t

            def load_xT(handle):  # [D, S] -> [128, KC, S]
                t = xpool.tile([P, KC, S], BF16, tag="xslot")
                dma(nc.sync, t, handle.rearrange("(c p) t -> p c t", p=P))
                return t

            # loads ordered so SA's first matmuls can start early
            saq = load_w("saq", w_saqT, KC, D)
            bsaq = load_b("bsaq", b_saq, KC)
            xqo_sb = singles.tile([P, KC, T], BF16, tag="xqo")
            dma(nc.sync, xqo_sb, xqT_own.rearrange("(c p) t -> p c t", p=P))
            sak = load_w("sak", w_sakT, KC, D)
            bsak = load_b("bsak", b_sak, KC)
            xq_sb = load_xT(xqT_full)
            sav = load_w("sav", w_savT, KC, D)
            mask_sb = mpool.tile([P, TC, T], BF16, tag="mslot")
            dma(nc.sync, mask_sb, maskT.rearrange("(c p) q -> p c q", p=P))
            xq_tm = tmpool.tile([P, QC, D], F32, tag="tmslot")
            dma(nc.sync, xq_tm, xq_own_tm.rearrange("(c p) d -> p c d", p=P))
            mq = load_w("mq", w_mqT, KC, D)
            mk = load_w("mk", w_mkT, KC, D)
            mv = load_w("mv", w_mvT, KC, D)
            mo = load_w("mo", w_moT, KC, D)
            bmq = load_b("bmq", b_mq, KC)
            bmk = load_b("bmk", b_mk, KC)
            bf1 = load_b("bf1", b_f1, FFC)
            l1g = load_b("l1g", b_l1g, KC)
            l1b = load_b("l1b", b_l1b, KC)
            l2g = load_b("l2g", b_l2g, KC)
            l2b = load_b("l2b", b_l2b, KC)
            x1o_sb = singles.tile([P, KC, T], BF16, tag="x1o")
            dma(nc.sync, x1o_sb, x1T_own.rearrange("(c p) t -> p c t", p=P))
            x2o_sb = singles.tile([P, KC, T], BF16, tag="x2o")
            dma(nc.sync, x2o_sb, x2T_own.rearrange("(c p) t -> p c t", p=P))

            # ---- helpers ----
            def proj_fm(wt, xt, bias, ntok, name):
                """OT fm [128, KC, ntok] bf16 = wt.T-style proj of xt + bias."""
                dst = qpool.tile([P, KC, ntok], BF16, tag=f"prj_{ntok}")
                for oc in range(KC):
                    for nk in range(ntok // 512):
                        ps = psA.tile([P, 512], F32, tag="big")
                        for kc in range(KC):
                            nc.tensor.matmul(
                                ps,
                                wt[:, kc, oc * P:(oc + 1) * P],
                                xt[:, kc, nk * 512:(nk + 1) * 512],
                                start=(kc == 0),
                                stop=(kc == KC - 1),
                            )
                        if bias is not None:
                            nc.vector.tensor_scalar_add(
                                dst[:, oc, nk * 512:(nk + 1) * 512],
                                ps, bias[:, oc:oc + 1])
                        else:
                            nc.vector.tensor_copy(
                                dst[:, oc, nk * 512:(nk + 1) * 512], ps)
                return dst

            def proj_v_tm(wt, xt):
                """V token-major with ones column: [128, TC, 8*65] bf16."""
                v = vpool.tile([P, TC, H * (DK + 1)], BF16, tag="vslot")
                v4 = v.rearrange("p t (h w) -> p t h w", w=DK + 1)
                nc.vector.memset(v4[:, :, :, DK:DK + 1], 1.0)
                for tcx in range(TC):
                    ps = psA.tile([P, 512], F32, tag="big")
                    for kc in range(KC):
                        nc.tensor.matmul(
                            ps,
                            xt[:, kc, tcx * P:(tcx + 1) * P],
                            wt[:, kc, :],
                            start=(kc == 0),
                            stop=(kc == KC - 1),
                        )
                    nc.vector.tensor_copy(
                        v4[:, tcx, :, 0:DK],
                        ps.rearrange("p (h w) -> p h w", w=DK),
                    )
                return v

            def tp_kv_out(xo_sb, ki, ko, vi, vo):
                """Project own-token K/V slices, stage to DRAM, AllGather
                across the 4-core batch group."""
                k_own = tppool.tile([P, KC, T], BF16, tag="tpk")
                for oc in range(KC):
                    ps = psC.tile([P, 512], F32, tag="tr")
                    for kc in range(KC):
                        nc.tensor.matmul(
                            ps, mk[:, kc, oc * P:(oc + 1) * P],
                            xo_sb[:, kc, :],
                            start=(kc == 0), stop=(kc == KC - 1))
                    nc.vector.tensor_scalar_add(k_own[:, oc, :], ps,
                                                bmk[:, oc:oc + 1])
                dk = dma(nc.sync, ki.rearrange("(c p) t -> p c t", p=P), k_own)
                ck = nc.gpsimd.collective_compute(
                    kind="AllGather", op=ALU.bypass, replica_groups=GROUPS,
                    ins=[ki[:]], outs=[ko[:]])
                add_dep_helper(ck.ins, dk.ins, sync=True, reason="cc in")
                v_own = tppool.tile([P, QC, VW], BF16, tag="tpv")
                v4 = v_own.rearrange("p t (h w) -> p t h w", w=DK + 1)
                nc.vector.memset(v4[:, :, :, DK:DK + 1], 1.0)
                for tcl in range(QC):
                    ps = psC.tile([P, 512], F32, tag="tr")
                    for kc in range(KC):
                        nc.tensor.matmul(
                            ps, xo_sb[:, kc, tcl * P:(tcl + 1) * P],
                            mv[:, kc, :],
                            start=(kc == 0), stop=(kc == KC - 1))
                    nc.vector.tensor_copy(
                        v4[:, tcl, :, 0:DK],
                        ps.rearrange("p (h w) -> p h w", w=DK))
                dv = dma(nc.sync, vi.rearrange("(c p) w -> p c w", p=P), v_own)
                cv = nc.gpsimd.collective_compute(
                    kind="AllGather", op=ALU.bypass, replica_groups=GROUPS,
                    ins=[vi[:]], outs=[vo[:]])
                add_dep_helper(cv.ins, dv.ins, sync=True, reason="cc in")
                return ck, cv

            def tp_kv_in(ko, vo, ck, cv):
                kT_full = kpool.tile([P, KC, S], BF16, tag="kslot",
                                     name="ktf")
                v_full = vpool.tile([P, TC, VW], BF16, tag="vslot",
                                    name="vtf")
                for m in range(4):
                    dk = dma(nc.sync, kT_full[:, :, m * T:(m + 1) * T],
                             ko[m].rearrange("(c p) t -> p c t", p=P))
                    add_dep_helper(dk.ins, ck.ins, sync=True, reason="cc out")
                    dv = dma(nc.sync, v_full[:, m * QC:(m + 1) * QC, :],
                             vo[m].rearrange("(c p) w -> p c w", p=P))
                    add_dep_helper(dv.ins, cv.ins, sync=True, reason="cc out")
                return kT_full, v_full

            def attention(qT, kT, v, masked, bg=None, tail_bg=None):
                """-> attn_tm [128, QC, D] f32, normalized (no v-bias yet).

                Head-pair loop: the two half-array (K=64) score matmuls of a
                pair target row groups 0:64 / 64:128 and disjoint bank halves
                of one 2-bank PSUM tile, so they run concurrently; one wide
                EXP covers both heads."""
                attn = tmpool.tile([P, QC, D], F32, tag="tmslot")
                for j in range(H // 2):
                    h0, h1 = 2 * j, 2 * j + 1
                    pa0 = psB.tile([DK + 1, 512], F32, tag="av")
                    pa1 = psB.tile([DK + 1, 512], F32, tag="av")
                    ets = {}
                    LAG = 3
                    for tcx in range(TC + LAG):
                        if tcx < TC:
                            ps = psA.tile([P, 1024], F32, tag="big")
                            nc.tensor.matmul(
                                ps[:, 0:512],
                                kT[0:DK, j, tcx * P:(tcx + 1) * P],
                                qT[0:DK, j, :], start=True, stop=True)
                            nc.tensor.matmul(
                                ps[:, 512:1024],
                                kT[DK:2 * DK, j, tcx * P:(tcx + 1) * P],
                                qT[DK:2 * DK, j, :], start=True, stop=True)
                            et = epool.tile([P, 1024], BF16, tag="exp")
                            nc.scalar.activation(et, ps, AF.Exp)
                            if masked:
                                nc.vector.tensor_tensor(
                                    et[:, 0:512], et[:, 0:512],
                                    mask_sb[:, tcx, :], ALU.mult)
                                nc.vector.tensor_tensor(
                                    et[:, 512:1024], et[:, 512:1024],
                                    mask_sb[:, tcx, :], ALU.mult)
                            ets[tcx] = et
                        if tcx >= LAG:
                            t0 = tcx - LAG
                            et = ets.pop(t0)
                            nc.tensor.matmul(
                                pa0,
                                v[:, t0, h0 * (DK + 1):(h0 + 1) * (DK + 1)],
                                et[:, 0:512],
                                start=(t0 == 0), stop=(t0 == TC - 1))
                            nc.tensor.matmul(
                                pa1,
                                v[:, t0, h1 * (DK + 1):(h1 + 1) * (DK + 1)],
                                et[:, 512:1024],
                                start=(t0 == 0), stop=(t0 == TC - 1))
                        if bg and (tcx % 2 == 0 or len(bg) > 24):
                            bg.pop(0)()
                    for h, pa in ((h0, pa0), (h1, pa1)):
                        av = apool.tile([DK + 1, 512], F32, tag="avsb")
                        nc.vector.tensor_copy(av, pa)
                        for qcx in range(QC):
                            pt = psC.tile([P, DK + 1], F32, tag="tr")
                            nc.tensor.transpose(
                                pt, av[:, qcx * P:(qcx + 1) * P],
                                id_f32[:DK + 1, :DK + 1])
                            rc = stat.tile([P, 1], F32, tag="rc")
                            nc.vector.reciprocal(rc, pt[:, DK:DK + 1])
                            nc.vector.tensor_scalar_mul(
                                attn[:, qcx, h * DK:(h + 1) * DK],
                                pt[:, 0:DK], rc)
                while bg:
                    bg.pop(0)()
                # units that write tiles whose slot is released only by this
                # attention's final AV matmuls (emitting them earlier wedges
                # the engine queues behind the WAR wait)
                while tail_bg:
                    tail_bg.pop(0)()
                return attn

            def ln_stats(mvall, z, qcx):
                st = stat.tile([P, nc.vector.BN_STATS_DIM], F32, tag="st")
                nc.vector.bn_stats(st, z[:, qcx, :])
                nc.vector.bn_aggr(mvall[:, qcx, :], st)

            def ln_apply(mvall, z, g_row, b_row, gb_pk=None, out_ap=None):
                """Normalize z in place given precomputed stats. If gb_pk is
                given, also emit the fm transpose per query-chunk: transposes
                read the pre-affine normalized z (no wait on the tm affine),
                and g/b are applied per-partition on the psum->fm copy."""
                g_bc = bcast_row(g_row)
                b_bc = bcast_row(b_row)
                if gb_pk is not None:
                    dst = fmpool.tile([P, KC, T], BF16, tag="fm", name="fmt")
                    g_pk, b_pk = gb_pk
                else:
                    dst = None
                for qcx in range(QC):
                    sd = stat.tile([P, 1], F32, tag="sds")
                    nc.scalar.activation(sd, mvall[:, qcx, 1:2], AF.Sqrt,
                                         bias=eps_sb, scale=1.0)
                    rstd = stat.tile([P, 1], F32, tag="rstds")
                    nc.vector.reciprocal(rstd, sd)
                    nc.vector.tensor_scalar(
                        z[:, qcx, :], z[:, qcx, :],
                        mvall[:, qcx, 0:1], rstd,
                        op0=ALU.subtract, op1=ALU.mult)
                    if dst is not None:
                        for dc in range(KC):
                            pt = psC.tile([P, P], F32, tag="tr")
                            nc.tensor.transpose(
                                pt, z[:, qcx, dc * P:(dc + 1) * P], id_f32)
                            nc.vector.tensor_scalar(
                                dst[:, dc, qcx * P:(qcx + 1) * P], pt,
                                g_pk[:, dc:dc + 1], b_pk[:, dc:dc + 1],
                                op0=ALU.mult, op1=ALU.add)
                    nc.vector.tensor_tensor(z[:, qcx, :], z[:, qcx, :], g_bc,
                                            ALU.mult)
                    nc.vector.tensor_add(z[:, qcx, :], z[:, qcx, :], b_bc)
                    if out_ap is not None:
                        dma(nc.sync, out_ap[:, qcx, :], z[:, qcx, :])
                return dst

            def transpose_tm_to_fm(src_tm):
                dst = fmpool.tile([P, KC, T], BF16, tag="fm")
                for qcx in range(QC):
                    for dc in range(KC):
                        pt = psC.tile([P, P], F32, tag="tr")
                        nc.tensor.transpose(
                            pt, src_tm[:, qcx, dc * P:(dc + 1) * P], id_f32)
                        nc.vector.tensor_copy(
                            dst[:, dc, qcx * P:(qcx + 1) * P], pt)
                return dst

            # ======== self-attention ========
            qT = proj_fm(saq, xqo_sb, bsaq, T, "saQ")
            kT = kpool.tile([P, KC, S], BF16, tag="kslot")
            for oc in range(KC):
                for nk in range(S // 512):
                    ps = psA.tile([P, 512], F32, tag="big")
                    for kc in range(KC):
                        nc.tensor.matmul(
                            ps, sak[:, kc, oc * P:(oc + 1) * P],
                            xq_sb[:, kc, nk * 512:(nk + 1) * 512],
                            start=(kc == 0), stop=(kc == KC - 1))
                    nc.vector.tensor_scalar_add(
                        kT[:, oc, nk * 512:(nk + 1) * 512], ps,
                        bsak[:, oc:oc + 1])
            v = proj_v_tm(sav, xq_sb)

            # TP: project own K/V slices for both cross-attns, AllGather
            ck1, cv1 = tp_kv_out(x1o_sb, st["k1i"], st["k1o"],
                                 st["v1i"], st["v1o"])
            kT_ca1, v_ca1 = tp_kv_in(st["k1o"], st["v1o"], ck1, cv1)

            attn = attention(qT, kT, v, masked=True)
            # second TP projection emitted here: its 32 matmuls fill the
            # PE gap while the SA->CA1 LayerNorm chain runs on DVE
            ck2, cv2 = tp_kv_out(x2o_sb, st["k2i"], st["k2o"],
                                 st["v2i"], st["v2o"])
            bv_bc = bcast_row(rows["bv_sa"])
            z1 = rpool.tile([P, QC, D], F32, tag="resid")
            mv1 = stat.tile([P, QC, 2], F32, tag="mvall")
            for qcx in range(QC):
                nc.vector.tensor_add(attn[:, qcx, :], attn[:, qcx, :], bv_bc)
                nc.vector.tensor_tensor(z1[:, qcx, :], xq_tm[:, qcx, :],
                                        attn[:, qcx, :], ALU.add)
                ln_stats(mv1, z1, qcx)
            yT1 = ln_apply(mv1, z1, rows["ln1g"], rows["ln1b"],
                           gb_pk=(l1g, l1b))

            # ======== cross-attention block (shared weights), used twice ====
            def cross_block(y_tm, yT, kTc, vc, bg, tail_bg, last=False):
                qTc = proj_fm(mq, yT, bmq, T, "q")
                attnc = attention(qTc, kTc, vc, masked=False, bg=bg,
                                  tail_bg=tail_bg)
                bvm_bc = bcast_row(rows["bv_m"])
                for qcx in range(QC):
                    nc.vector.tensor_add(attnc[:, qcx, :], attnc[:, qcx, :],
                                         bvm_bc)
                attnT = transpose_tm_to_fm(attnc)
                # out-projection directly token-major:
                # psum[tok, outD] = sum_kc attnT[:,kc,tok-chunk].T @ moT[:,kc,:]
                bmo_bc = bcast_row(rows["bmo"])
                z = rpool.tile([P, QC, D], F32, tag="resid")
                mvc = stat.tile([P, QC, 2], F32, tag="mvall")
                for qcx in range(QC):
                    ps = psA.tile([P, 512], F32, tag="big")
                    for kc in range(KC):
                        nc.tensor.matmul(
                            ps, attnT[:, kc, qcx * P:(qcx + 1) * P],
                            mo[:, kc, :],
                            start=(kc == 0), stop=(kc == KC - 1))
                    nc.vector.tensor_tensor(z[:, qcx, :], ps, y_tm[:, qcx, :],
                                            ALU.add)
                    nc.vector.tensor_add(z[:, qcx, :], z[:, qcx, :], bmo_bc)
                    ln_stats(mvc, z, qcx)
                zT = ln_apply(mvc, z, rows["ln2g"], rows["ln2b"],
                              gb_pk=(l2g, l2b))
                return z, zT

            kT_ca2, v_ca2 = tp_kv_in(st["k2o"], st["v2o"], ck2, cv2)
            y1, yT2 = cross_block(z1, yT1, kT_ca1, v_ca1, None, None)
            y2, y2T = cross_block(y1, yT2, kT_ca2, v_ca2, None, None)

            # ======== FFN ========
            w1 = xpool.tile([P, KC, FF], BF16, tag="xslot")
            dma(nc.sync, w1, w_f1T.rearrange("(c p) o -> p c o", p=P))
            w2 = kpool.tile([P, FFC, D], BF16, tag="kslot")
            dma(nc.sync, w2, w_f2T.rearrange("(c p) o -> p c o", p=P))

            hT = mpool.tile([P, FFC, T], BF16, tag="mslot")
            for oc in range(FFC):
                ps = psA.tile([P, 512], F32, tag="big")
                for kc in range(KC):
                    nc.tensor.matmul(
                        ps, w1[:, kc, oc * P:(oc + 1) * P], y2T[:, kc, :],
                        start=(kc == 0), stop=(kc == KC - 1))
                nc.scalar.activation(hT[:, oc, :], ps, AF.Relu,
                                     bias=bf1[:, oc:oc + 1], scale=1.0)
            bf2_bc = bcast_row(rows["bf2"])
            z3 = rpool.tile([P, QC, D], F32, tag="resid")
            mv3 = stat.tile([P, QC, 2], F32, tag="mvall")
            for qcx in range(QC):
                ps = psA.tile([P, 512], F32, tag="big")
                for kc in range(FFC):
                    nc.tensor.matmul(
                        ps, hT[:, kc, qcx * P:(qcx + 1) * P], w2[:, kc, :],
                        start=(kc == 0), stop=(kc == FFC - 1))
                nc.vector.tensor_tensor(z3[:, qcx, :], ps, y2[:, qcx, :],
                                        ALU.add)
                nc.vector.tensor_add(z3[:, qcx, :], z3[:, qcx, :], bf2_bc)
                ln_stats(mv3, z3, qcx)
            ln_apply(mv3, z3, rows["ln3g"], rows["ln3b"],
                     out_ap=out.rearrange("(c p) d -> p c d", p=P))

    nc.finalize()
    return nc


_NC_CACHE = None


def _prep_inputs(x_q, x1, x2, sa_wq, sa_bq, sa_wk, sa_bk, sa_wv, sa_bv,
                 ln1_g, ln1_b, mha_in_w, mha_in_b, mha_out_w, mha_out_b,
                 ln2_g, ln2_b, ffn_w1, ffn_b1, ffn_w2, ffn_b2, ln3_g, ln3_b):
    bf = ml_dtypes.bfloat16
    f32 = np.float32
    scale = 1.0 / np.sqrt(np.float32(DK))

    def pk(b):  # [O] -> [128, O//128] per-partition packed
        return np.ascontiguousarray(np.asarray(b, f32).reshape(-1, P).T)

    shared = {
        "w_saqT": np.ascontiguousarray((sa_wq * scale).T.astype(bf)),
        "w_sakT": np.ascontiguousarray(sa_wk.T.astype(bf)),
        "w_savT": np.ascontiguousarray(sa_wv.T.astype(bf)),
        "w_mqT": np.ascontiguousarray((mha_in_w[:D] * scale).T.astype(bf)),
        "w_mkT": np.ascontiguousarray(mha_in_w[D:2 * D].T.astype(bf)),
        "w_mvT": np.ascontiguousarray(mha_in_w[2 * D:].T.astype(bf)),
        "w_moT": np.ascontiguousarray(mha_out_w.T.astype(bf)),
        "w_f1T": np.ascontiguousarray(ffn_w1.T.astype(bf)),
        "w_f2T": np.ascontiguousarray(ffn_w2.T.astype(bf)),
        "b_saq": pk(sa_bq * scale),
        "b_sak": pk(sa_bk),
        "b_mq": pk(mha_in_b[:D] * scale),
        "b_mk": pk(mha_in_b[D:2 * D]),
        "b_f1": pk(ffn_b1),
        "b_l1g": pk(ln1_g), "b_l1b": pk(ln1_b),
        "b_l2g": pk(ln2_g), "b_l2b": pk(ln2_b),
        "bv_sa": np.asarray(sa_bv, f32),
        "bv_m": np.asarray(mha_in_b[2 * D:], f32),
        "bmo": np.asarray(mha_out_b, f32),
        "bf2": np.asarray(ffn_b2, f32),
        "ln1g": np.asarray(ln1_g, f32), "ln1b": np.asarray(ln1_b, f32),
        "ln2g": np.asarray(ln2_g, f32), "ln2b": np.asarray(ln2_b, f32),
        "ln3g": np.asarray(ln3_g, f32), "ln3b": np.asarray(ln3_b, f32),
    }

    kk = np.arange(S, dtype=np.int64)[:, None]
    in_maps = []
    for c in range(NC):
        b, s = c // 4, c % 4
        xT = np.ascontiguousarray(x_q[b].T.astype(bf))      # [D, S]
        qq = np.arange(T, dtype=np.int64)[None, :] + s * T
        m = np.where(kk <= qq, 1.0, 0.0).astype(bf)         # [S, T] 0/1 mult mask
        im = dict(shared)
        im["xqT_full"] = xT
        im["xqT_own"] = np.ascontiguousarray(xT[:, s * T:(s + 1) * T])
        im["x1T_own"] = np.ascontiguousarray(
            x1[b].T[:, s * T:(s + 1) * T].astype(bf))
        im["x2T_own"] = np.ascontiguousarray(
            x2[b].T[:, s * T:(s + 1) * T].astype(bf))
        im["xq_own_tm"] = np.ascontiguousarray(
            x_q[b, s * T:(s + 1) * T, :].astype(f32))
        im["maskT"] = m
        in_maps.append(im)
    return in_maps


def kernel(**inputs):
    global _NC_CACHE
    if _NC_CACHE is None:
        _NC_CACHE = build_nc()
    nc = _NC_CACHE
    in_maps = _prep_inputs(**{k: np.asarray(v) for k, v in inputs.items()})
    res = run_bass_kernel_spmd(nc, in_maps, core_ids=list(range(NC)))
    full = np.empty((B, S, D), np.float32)
    for c in range(NC):
        b, s = c // 4, c % 4
        full[b, s * T:(s + 1) * T, :] = res.results[c]["out"]
    return full


# revision 90
# speedup vs baseline: 1.0446x; 1.0293x over previous
"""Trainium2 Bass kernel for nn_EnrichBlock (B=2,S=2048,D=512,H=8,FF=2048).

Sharding: token-parallel over (batch, seq) -> 8 shards of 512 query tokens.
No collectives: each core recomputes K/V projections for its batch element
from the raw inputs (all cross-token deps flow through raw x_q/x1/x2).

Host prep: weights pre-transposed to [in,out] (Q-side pre-scaled by 1/8),
activations pre-transposed to feature-major [D, T], per-core causal mask in
key-major layout. All device DMAs are then contiguous.

On-chip: bf16 matmul operands, fp32 PSUM/LN/softmax-denominators. Attention
is computed key-major (scoresT[k,q] = K_h^T-chunk x Q_h), causal mask is
PSUM-injected via an identity matmul, exp on ACT, softmax denominator via an
appended ones-column in V, then a per-head PE transpose back to token-major
with the denominator riding along as a 65th row; normalization, head-merge,
residual and LayerNorm all happen token-major.
"""

import numpy as np
import ml_dtypes

import concourse.bass as bass
import concourse.mybir as mybir
import concourse.tile as tile
from concourse.bacc import Bacc
from concourse.masks import make_identity
from concourse.tile_rust import add_dep_helper
from concourse.bass_utils import run_bass_kernel_spmd

B, S, D, H, FF = 2, 2048, 512, 8, 2048
DK = D // H          # 64
T = 512              # query tokens per core
NC = 8               # cores
EPS = 1e-5
P = 128
F32 = mybir.dt.float32
BF16 = mybir.dt.bfloat16
AF = mybir.ActivationFunctionType
ALU = mybir.AluOpType

KC = D // P          # 4   contraction chunks over D
QC = T // P          # 4   query-token chunks
TC = S // P          # 16  key-token chunks
FFC = FF // P        # 16


def build_nc():
    nc = Bacc(num_devices=NC)

    def dma(engine, dst, src):
        return engine.dma_start(out=dst, in_=src)

    # ---- DRAM tensors ----
    di = lambda n, sh, dt: nc.dram_tensor(n, sh, dt, kind="ExternalInput")
    xqT_own = di("xqT_own", [D, T], BF16)       # feature-major own slice
    xqT_full = di("xqT_full", [D, S], BF16)     # feature-major full batch elem
    x1T_own = di("x1T_own", [D, T], BF16)       # own token slice only (TP)
    x2T_own = di("x2T_own", [D, T], BF16)
    # staging for the 4-way K/V AllGathers (tensor-parallel projections)
    VW = H * (DK + 1)
    st = {}
    for nm in ["k1", "v1", "k2", "v2"]:
        inner = [D, T] if nm[0] == "k" else [T, VW]
        st[nm + "i"] = nc.dram_tensor(nm + "i", inner, BF16, kind="Internal")
        st[nm + "o"] = nc.dram_tensor(nm + "o", [4] + inner, BF16,
                                      kind="Internal")
    GROUPS = [[0, 1, 2, 3], [4, 5, 6, 7]]
    xq_own_tm = di("xq_own_tm", [T, D], F32)    # token-major own slice
    maskT = di("maskT", [S, T], BF16)           # additive causal mask, key-major

    w_saqT = di("w_saqT", [D, D], BF16)         # sa_wq.T / 8
    w_sakT = di("w_sakT", [D, D], BF16)
    w_savT = di("w_savT", [D, D], BF16)
    w_mqT = di("w_mqT", [D, D], BF16)           # mha Wq.T / 8
    w_mkT = di("w_mkT", [D, D], BF16)
    w_mvT = di("w_mvT", [D, D], BF16)
    w_moT = di("w_moT", [D, D], BF16)
    w_f1T = di("w_f1T", [D, FF], BF16)
    w_f2T = di("w_f2T", [FF, D], BF16)

    b_saq = di("b_saq", [P, KC], F32)           # packed per-partition biases
    b_sak = di("b_sak", [P, KC], F32)
    b_mq = di("b_mq", [P, KC], F32)
    b_mk = di("b_mk", [P, KC], F32)
    b_f1 = di("b_f1", [P, FFC], F32)
    b_l1g = di("b_l1g", [P, KC], F32)
    b_l1b = di("b_l1b", [P, KC], F32)
    b_l2g = di("b_l2g", [P, KC], F32)
    b_l2b = di("b_l2b", [P, KC], F32)

    # row vectors, broadcast over partitions at load time
    rows = {}
    for n in ["bv_sa", "bv_m", "bmo", "bf2",
              "ln1g", "ln1b", "ln2g", "ln2b", "ln3g", "ln3b"]:
        rows[n] = di(n, [D], F32)

    out = nc.dram_tensor("out", [T, D], F32, kind="ExternalOutput")

    with tile.TileContext(nc) as tc:
        with (
            tc.tile_pool(name="singles", bufs=1) as singles,
            tc.tile_pool(name="xpool", bufs=2) as xpool,      # 2 x 2MB slots
            tc.tile_pool(name="kpool", bufs=2) as kpool,      # kT / w2T slots
            tc.tile_pool(name="vpool", bufs=1) as vpool,
            tc.tile_pool(name="mpool", bufs=1) as mpool,      # maskT / hT slot
            tc.tile_pool(name="qpool", bufs=1) as qpool,
            tc.tile_pool(name="epool", bufs=6) as epool,
            tc.tile_pool(name="apool", bufs=2) as apool,
            tc.tile_pool(name="tmpool", bufs=2) as tmpool,
            tc.tile_pool(name="rpool", bufs=2) as rpool,
            tc.tile_pool(name="tppool", bufs=1) as tppool,
            tc.tile_pool(name="fmpool", bufs=1) as fmpool,
            tc.tile_pool(name="bcpool", bufs=2) as bcpool,
            tc.tile_pool(name="stat", bufs=6) as stat,
            tc.tile_pool(name="psA", bufs=2, space="PSUM") as psA,
            tc.tile_pool(name="psB", bufs=2, space="PSUM") as psB,
            tc.tile_pool(name="psC", bufs=2, space="PSUM") as psC,
        ):
            # ---- constants / persistent loads ----
            id_f32 = singles.tile([P, P], F32, tag="id_f32")
            make_identity(nc, id_f32)
            eps_sb = singles.tile([P, 1], F32, tag="eps")
            nc.vector.memset(eps_sb, EPS)

            def load_w(name, t, icnk, ocols):  # [I,O] dram -> [128, icnk, ocols]
                w = singles.tile([P, icnk, ocols], BF16, tag=name)
                dma(nc.sync, w, t.rearrange("(c p) o -> p c o", p=P))
                return w

            def load_b(name, t, cols):
                b = singles.tile([P, cols], F32, tag=name)
                dma(nc.sync, b, t[:, :])
                return b

            def bcast_row(handle):  # [D] dram row -> [128, D] sbuf
                t = bcpool.tile([P, D], F32, tag="bc")
                src = bass.AP(
                    tensor=handle[:].tensor,
                    offset=handle[:].offset,
                    ap=[[0, P], [1, D]],
                )
                dma(nc.gpsimd, t, src)
                return t

            def load_xT(handle):  # [D, S] -> [128, KC, S]
                t = xpool.tile([P, KC, S], BF16, tag="xslot")
                dma(nc.sync, t, handle.rearrange("(c p) t -> p c t", p=P))
                return t

            # loads ordered so SA's first matmuls can start early
            saq = load_w("saq", w_saqT, KC, D)
            bsaq = load_b("bsaq", b_saq, KC)
            xqo_sb = singles.tile([P, KC, T], BF16, tag="xqo")
            dma(nc.sync, xqo_sb, xqT_own.rearrange("(c p) t -> p c t", p=P))
            sak = load_w("sak", w_sakT, KC, D)
            bsak = load_b("bsak", b_sak, KC)
            xq_sb = load_xT(xqT_full)
            sav = load_w("sav", w_savT, KC, D)
            mask_sb = mpool.tile([P, TC, T], BF16, tag="mslot")
            dma(nc.sync, mask_sb, maskT.rearrange("(c p) q -> p c q", p=P))
            xq_tm = tmpool.tile([P, QC, D], F32, tag="tmslot")
            dma(nc.sync, xq_tm, xq_own_tm.rearrange("(c p) d -> p c d", p=P))
            mq = load_w("mq", w_mqT, KC, D)
            mk = load_w("mk", w_mkT, KC, D)
            mv = load_w("mv", w_mvT, KC, D)
            mo = load_w("mo", w_moT, KC, D)
            bmq = load_b("bmq", b_mq, KC)
            bmk = load_b("bmk", b_mk, KC)
            bf1 = load_b("bf1", b_f1, FFC)
            l1g = load_b("l1g", b_l1g, KC)
            l1b = load_b("l1b", b_l1b, KC)
            l2g = load_b("l2g", b_l2g, KC)
            l2b = load_b("l2b", b_l2b, KC)
            x1o_sb = singles.tile([P, KC, T], BF16, tag="x1o")
            dma(nc.sync, x1o_sb, x1T_own.rearrange("(c p) t -> p c t", p=P))
            x2o_sb = singles.tile([P, KC, T], BF16, tag="x2o")
            dma(nc.sync, x2o_sb, x2T_own.rearrange("(c p) t -> p c t", p=P))

            # ---- helpers ----
            def proj_fm(wt, xt, bias, ntok, name, nkw=512):
                """OT fm [128, KC, ntok] bf16 = wt.T-style proj of xt + bias.

                nkw < 512 lets the projection start before all of xt's
                token chunks exist (e.g. right after half the LN transposes)."""
                dst = qpool.tile([P, KC, ntok], BF16, tag=f"prj_{ntok}")
                for nk in range(ntok // nkw):
                    for oc in range(KC):
                        ps = psA.tile([P, nkw], F32, tag="big")
                        for kc in range(KC):
                            nc.tensor.matmul(
                                ps,
                                wt[:, kc, oc * P:(oc + 1) * P],
                                xt[:, kc, nk * nkw:(nk + 1) * nkw],
                                start=(kc == 0),
                                stop=(kc == KC - 1),
                            )
                        if bias is not None:
                            nc.vector.tensor_scalar_add(
                                dst[:, oc, nk * nkw:(nk + 1) * nkw],
                                ps, bias[:, oc:oc + 1])
                        else:
                            nc.vector.tensor_copy(
                                dst[:, oc, nk * nkw:(nk + 1) * nkw], ps)
                return dst

            def proj_v_tm(wt, xt):
                """V token-major with ones column: [128, TC, 8*65] bf16."""
                v = vpool.tile([P, TC, H * (DK + 1)], BF16, tag="vslot")
                v4 = v.rearrange("p t (h w) -> p t h w", w=DK + 1)
                nc.vector.memset(v4[:, :, :, DK:DK + 1], 1.0)
                for tcx in range(TC):
                    ps = psA.tile([P, 512], F32, tag="big")
                    for kc in range(KC):
                        nc.tensor.matmul(
                            ps,
                            xt[:, kc, tcx * P:(tcx + 1) * P],
                            wt[:, kc, :],
                            start=(kc == 0),
                            stop=(kc == KC - 1),
                        )
                    nc.vector.tensor_copy(
                        v4[:, tcx, :, 0:DK],
                        ps.rearrange("p (h w) -> p h w", w=DK),
                    )
                return v

            def tp_kv_out(xo_sb, ki, ko, vi, vo):
                """Project own-token K/V slices, stage to DRAM, AllGather
                across the 4-core batch group."""
                k_own = tppool.tile([P, KC, T], BF16, tag="tpk")
                for oc in range(KC):
                    ps = psC.tile([P, 512], F32, tag="tr")
                    for kc in range(KC):
                        nc.tensor.matmul(
                            ps, mk[:, kc, oc * P:(oc + 1) * P],
                            xo_sb[:, kc, :],
                            start=(kc == 0), stop=(kc == KC - 1))
                    nc.vector.tensor_scalar_add(k_own[:, oc, :], ps,
                                                bmk[:, oc:oc + 1])
                dk = dma(nc.sync, ki.rearrange("(c p) t -> p c t", p=P), k_own)
                ck = nc.gpsimd.collective_compute(
                    kind="AllGather", op=ALU.bypass, replica_groups=GROUPS,
                    ins=[ki[:]], outs=[ko[:]])
                add_dep_helper(ck.ins, dk.ins, sync=True, reason="cc in")
                v_own = tppool.tile([P, QC, VW], BF16, tag="tpv")
                v4 = v_own.rearrange("p t (h w) -> p t h w", w=DK + 1)
                nc.vector.memset(v4[:, :, :, DK:DK + 1], 1.0)
                for tcl in range(QC):
                    ps = psC.tile([P, 512], F32, tag="tr")
                    for kc in range(KC):
                        nc.tensor.matmul(
                            ps, xo_sb[:, kc, tcl * P:(tcl + 1) * P],
                            mv[:, kc, :],
                            start=(kc == 0), stop=(kc == KC - 1))
                    nc.vector.tensor_copy(
                        v4[:, tcl, :, 0:DK],
                        ps.rearrange("p (h w) -> p h w", w=DK))
                dv = dma(nc.sync, vi.rearrange("(c p) w -> p c w", p=P), v_own)
                cv = nc.gpsimd.collective_compute(
                    kind="AllGather", op=ALU.bypass, replica_groups=GROUPS,
                    ins=[vi[:]], outs=[vo[:]])
                add_dep_helper(cv.ins, dv.ins, sync=True, reason="cc in")
                return ck, cv

            def tp_kv_in(ko, vo, ck, cv):
                kT_full = kpool.tile([P, KC, S], BF16, tag="kslot",
                                     name="ktf")
                v_full = vpool.tile([P, TC, VW], BF16, tag="vslot",
                                    name="vtf")
                for m in range(4):
                    dk = dma(nc.sync, kT_full[:, :, m * T:(m + 1) * T],
                             ko[m].rearrange("(c p) t -> p c t", p=P))
                    add_dep_helper(dk.ins, ck.ins, sync=True, reason="cc out")
                    dv = dma(nc.sync, v_full[:, m * QC:(m + 1) * QC, :],
                             vo[m].rearrange("(c p) w -> p c w", p=P))
                    add_dep_helper(dv.ins, cv.ins, sync=True, reason="cc out")
                return kT_full, v_full

            def attention(qT, kT, v, masked, bg=None, tail_bg=None):
                """-> attn_tm [128, QC, D] f32, normalized (no v-bias yet).

                Head-pair loop: the two half-array (K=64) score matmuls of a
                pair target row groups 0:64 / 64:128 and disjoint bank halves
                of one 2-bank PSUM tile, so they run concurrently; one wide
                EXP covers both heads."""
                attn = tmpool.tile([P, QC, D], F32, tag="tmslot")
                for j in range(H // 2):
                    h0, h1 = 2 * j, 2 * j + 1
                    pa0 = psB.tile([DK + 1, 512], F32, tag="av")
                    pa1 = psB.tile([DK + 1, 512], F32, tag="av")
                    ets = {}
                    LAG = 3
                    for tcx in range(TC + LAG):
                        if tcx < TC:
                            ps = psA.tile([P, 1024], F32, tag="big")
                            nc.tensor.matmul(
                                ps[:, 0:512],
                                kT[0:DK, j, tcx * P:(tcx + 1) * P],
                                qT[0:DK, j, :], start=True, stop=True)
                            nc.tensor.matmul(
                                ps[:, 512:1024],
                                kT[DK:2 * DK, j, tcx * P:(tcx + 1) * P],
                                qT[DK:2 * DK, j, :], start=True, stop=True)
                            et = epool.tile([P, 1024], BF16, tag="exp")
                            nc.scalar.activation(et, ps, AF.Exp)
                            if masked:
                                nc.vector.tensor_tensor(
                                    et[:, 0:512], et[:, 0:512],
                                    mask_sb[:, tcx, :], ALU.mult)
                                nc.vector.tensor_tensor(
                                    et[:, 512:1024], et[:, 512:1024],
                                    mask_sb[:, tcx, :], ALU.mult)
                            ets[tcx] = et
                        if tcx >= LAG:
                            t0 = tcx - LAG
                            et = ets.pop(t0)
                            nc.tensor.matmul(
                                pa0,
                                v[:, t0, h0 * (DK + 1):(h0 + 1) * (DK + 1)],
                                et[:, 0:512],
                                start=(t0 == 0), stop=(t0 == TC - 1))
                            nc.tensor.matmul(
                                pa1,
                                v[:, t0, h1 * (DK + 1):(h1 + 1) * (DK + 1)],
                                et[:, 512:1024],
                                start=(t0 == 0), stop=(t0 == TC - 1))
                        if bg and (tcx % 2 == 0 or len(bg) > 24):
                            bg.pop(0)()
                    for h, pa in ((h0, pa0), (h1, pa1)):
                        av = apool.tile([DK + 1, 512], F32, tag="avsb")
                        nc.vector.tensor_copy(av, pa)
                        for qcx in range(QC):
                            pt = psC.tile([P, DK + 1], F32, tag="tr")
                            nc.tensor.transpose(
                                pt, av[:, qcx * P:(qcx + 1) * P],
                                id_f32[:DK + 1, :DK + 1])
                            rc = stat.tile([P, 1], F32, tag="rc")
                            nc.vector.reciprocal(rc, pt[:, DK:DK + 1])
                            nc.vector.tensor_scalar_mul(
                                attn[:, qcx, h * DK:(h + 1) * DK],
                                pt[:, 0:DK], rc)
                while bg:
                    bg.pop(0)()
                # units that write tiles whose slot is released only by this
                # attention's final AV matmuls (emitting them earlier wedges
                # the engine queues behind the WAR wait)
                while tail_bg:
                    tail_bg.pop(0)()
                return attn

            def ln_stats(mvall, z, qcx):
                st = stat.tile([P, nc.vector.BN_STATS_DIM], F32, tag="st")
                nc.vector.bn_stats(st, z[:, qcx, :])
                nc.vector.bn_aggr(mvall[:, qcx, :], st)

            def ln_apply(mvall, z, g_row, b_row, gb_pk=None, out_ap=None):
                """Normalize z in place given precomputed stats. If gb_pk is
                given, also emit the fm transpose per query-chunk: transposes
                read the pre-affine normalized z (no wait on the tm affine),
                and g/b are applied per-partition on the psum->fm copy."""
                g_bc = bcast_row(g_row)
                b_bc = bcast_row(b_row)
                if gb_pk is not None:
                    dst = fmpool.tile([P, KC, T], BF16, tag="fm", name="fmt")
                    g_pk, b_pk = gb_pk
                else:
                    dst = None
                for qcx in range(QC):
                    sd = stat.tile([P, 1], F32, tag="sds")
                    nc.scalar.activation(sd, mvall[:, qcx, 1:2], AF.Sqrt,
                                         bias=eps_sb, scale=1.0)
                    rstd = stat.tile([P, 1], F32, tag="rstds")
                    nc.vector.reciprocal(rstd, sd)
                    nc.vector.tensor_scalar(
                        z[:, qcx, :], z[:, qcx, :],
                        mvall[:, qcx, 0:1], rstd,
                        op0=ALU.subtract, op1=ALU.mult)
                    if dst is not None:
                        for dc in range(KC):
                            pt = psC.tile([P, P], F32, tag="tr")
                            nc.tensor.transpose(
                                pt, z[:, qcx, dc * P:(dc + 1) * P], id_f32)
                            nc.vector.tensor_scalar(
                                dst[:, dc, qcx * P:(qcx + 1) * P], pt,
                                g_pk[:, dc:dc + 1], b_pk[:, dc:dc + 1],
                                op0=ALU.mult, op1=ALU.add)
                    nc.vector.tensor_tensor(z[:, qcx, :], z[:, qcx, :], g_bc,
                                            ALU.mult)
                    nc.vector.tensor_add(z[:, qcx, :], z[:, qcx, :], b_bc)
                    if out_ap is not None:
                        dma(nc.sync, out_ap[:, qcx, :], z[:, qcx, :])
                return dst

            def transpose_tm_to_fm(src_tm):
                dst = fmpool.tile([P, KC, T], BF16, tag="fm")
                for qcx in range(QC):
                    for dc in range(KC):
                        pt = psC.tile([P, P], F32, tag="tr")
                        nc.tensor.transpose(
                            pt, src_tm[:, qcx, dc * P:(dc + 1) * P], id_f32)
                        nc.vector.tensor_copy(
                            dst[:, dc, qcx * P:(qcx + 1) * P], pt)
                return dst

            # ======== self-attention ========
            qT = proj_fm(saq, xqo_sb, bsaq, T, "saQ")
            kT = kpool.tile([P, KC, S], BF16, tag="kslot")
            for oc in range(KC):
                for nk in range(S // 512):
                    ps = psA.tile([P, 512], F32, tag="big")
                    for kc in range(KC):
                        nc.tensor.matmul(
                            ps, sak[:, kc, oc * P:(oc + 1) * P],
                            xq_sb[:, kc, nk * 512:(nk + 1) * 512],
                            start=(kc == 0), stop=(kc == KC - 1))
                    nc.vector.tensor_scalar_add(
                        kT[:, oc, nk * 512:(nk + 1) * 512], ps,
                        bsak[:, oc:oc + 1])
            v = proj_v_tm(sav, xq_sb)

            # TP: project own K/V slices for both cross-attns, AllGather
            ck1, cv1 = tp_kv_out(x1o_sb, st["k1i"], st["k1o"],
                                 st["v1i"], st["v1o"])
            kT_ca1, v_ca1 = tp_kv_in(st["k1o"], st["v1o"], ck1, cv1)

            attn = attention(qT, kT, v, masked=True)
            # second TP projection emitted here: its 32 matmuls fill the
            # PE gap while the SA->CA1 LayerNorm chain runs on DVE
            ck2, cv2 = tp_kv_out(x2o_sb, st["k2i"], st["k2o"],
                                 st["v2i"], st["v2o"])
            bv_bc = bcast_row(rows["bv_sa"])
            z1 = rpool.tile([P, QC, D], F32, tag="resid")
            mv1 = stat.tile([P, QC, 2], F32, tag="mvall")
            for qcx in range(QC):
                nc.vector.tensor_add(attn[:, qcx, :], attn[:, qcx, :], bv_bc)
                nc.vector.tensor_tensor(z1[:, qcx, :], xq_tm[:, qcx, :],
                                        attn[:, qcx, :], ALU.add)
                ln_stats(mv1, z1, qcx)
            yT1 = ln_apply(mv1, z1, rows["ln1g"], rows["ln1b"],
                           gb_pk=(l1g, l1b))

            # ======== cross-attention block (shared weights), used twice ====
            def cross_block(y_tm, yT, kTc, vc, bg, tail_bg, last=False):
                qTc = proj_fm(mq, yT, bmq, T, "q", nkw=256)
                attnc = attention(qTc, kTc, vc, masked=False, bg=bg,
                                  tail_bg=tail_bg)
                bvm_bc = bcast_row(rows["bv_m"])
                for qcx in range(QC):
                    nc.vector.tensor_add(attnc[:, qcx, :], attnc[:, qcx, :],
                                         bvm_bc)
                attnT = transpose_tm_to_fm(attnc)
                # out-projection directly token-major:
                # psum[tok, outD] = sum_kc attnT[:,kc,tok-chunk].T @ moT[:,kc,:]
                bmo_bc = bcast_row(rows["bmo"])
                z = rpool.tile([P, QC, D], F32, tag="resid")
                mvc = stat.tile([P, QC, 2], F32, tag="mvall")
                for qcx in range(QC):
                    ps = psA.tile([P, 512], F32, tag="big")
                    for kc in range(KC):
                        nc.tensor.matmul(
                            ps, attnT[:, kc, qcx * P:(qcx + 1) * P],
                            mo[:, kc, :],
                            start=(kc == 0), stop=(kc == KC - 1))
                    nc.vector.tensor_tensor(z[:, qcx, :], ps, y_tm[:, qcx, :],
                                            ALU.add)
                    nc.vector.tensor_add(z[:, qcx, :], z[:, qcx, :], bmo_bc)
                    ln_stats(mvc, z, qcx)
                zT = ln_apply(mvc, z, rows["ln2g"], rows["ln2b"],
                              gb_pk=(l2g, l2b))
                return z, zT

            kT_ca2, v_ca2 = tp_kv_in(st["k2o"], st["v2o"], ck2, cv2)
            y1, yT2 = cross_block(z1, yT1, kT_ca1, v_ca1, None, None)
            y2, y2T = cross_block(y1, yT2, kT_ca2, v_ca2, None, None)

            # ======== FFN ========
            w1 = xpool.tile([P, KC, FF], BF16, tag="xslot")
            dma(nc.sync, w1, w_f1T.rearrange("(c p) o -> p c o", p=P))
            w2 = kpool.tile([P, FFC, D], BF16, tag="kslot")
            dma(nc.sync, w2, w_f2T.rearrange("(c p) o -> p c o", p=P))

            hT = mpool.tile([P, FFC, T], BF16, tag="mslot")
            for oc in range(FFC):
                ps = psA.tile([P, 512], F32, tag="big")
                for kc in range(KC):
                    nc.tensor.matmul(
                        ps, w1[:, kc, oc * P:(oc + 1) * P], y2T[:, kc, :],
                        start=(kc == 0), stop=(kc == KC - 1))
                nc.scalar.activation(hT[:, oc, :], ps, AF.Relu,
                                     bias=bf1[:, oc:oc + 1], scale=1.0)
            bf2_bc = bcast_row(rows["bf2"])
            z3 = rpool.tile([P, QC, D], F32, tag="resid")
            mv3 = stat.tile([P, QC, 2], F32, tag="mvall")
            for qcx in range(QC):
                ps = psA.tile([P, 512], F32, tag="big")
                for kc in range(FFC):
                    nc.tensor.matmul(
                        ps, hT[:, kc, qcx * P:(qcx + 1) * P], w2[:, kc, :],
                        start=(kc == 0), stop=(kc == FFC - 1))
                nc.vector.tensor_tensor(z3[:, qcx, :], ps, y2[:, qcx, :],
                                        ALU.add)
                nc.vector.tensor_add(z3[:, qcx, :], z3[:, qcx, :], bf2_bc)
                ln_stats(mv3, z3, qcx)
            ln_apply(mv3, z3, rows["ln3g"], rows["ln3b"],
                     out_ap=out.rearrange("(c p) d -> p c d", p=P))

    nc.finalize()
    return nc


_NC_CACHE = None


def _prep_inputs(x_q, x1, x2, sa_wq, sa_bq, sa_wk, sa_bk, sa_wv, sa_bv,
                 ln1_g, ln1_b, mha_in_w, mha_in_b, mha_out_w, mha_out_b,
                 ln2_g, ln2_b, ffn_w1, ffn_b1, ffn_w2, ffn_b2, ln3_g, ln3_b):
    bf = ml_dtypes.bfloat16
    f32 = np.float32
    scale = 1.0 / np.sqrt(np.float32(DK))

    def pk(b):  # [O] -> [128, O//128] per-partition packed
        return np.ascontiguousarray(np.asarray(b, f32).reshape(-1, P).T)

    shared = {
        "w_saqT": np.ascontiguousarray((sa_wq * scale).T.astype(bf)),
        "w_sakT": np.ascontiguousarray(sa_wk.T.astype(bf)),
        "w_savT": np.ascontiguousarray(sa_wv.T.astype(bf)),
        "w_mqT": np.ascontiguousarray((mha_in_w[:D] * scale).T.astype(bf)),
        "w_mkT": np.ascontiguousarray(mha_in_w[D:2 * D].T.astype(bf)),
        "w_mvT": np.ascontiguousarray(mha_in_w[2 * D:].T.astype(bf)),
        "w_moT": np.ascontiguousarray(mha_out_w.T.astype(bf)),
        "w_f1T": np.ascontiguousarray(ffn_w1.T.astype(bf)),
        "w_f2T": np.ascontiguousarray(ffn_w2.T.astype(bf)),
        "b_saq": pk(sa_bq * scale),
        "b_sak": pk(sa_bk),
        "b_mq": pk(mha_in_b[:D] * scale),
        "b_mk": pk(mha_in_b[D:2 * D]),
        "b_f1": pk(ffn_b1),
        "b_l1g": pk(ln1_g), "b_l1b": pk(ln1_b),
        "b_l2g": pk(ln2_g), "b_l2b": pk(ln2_b),
        "bv_sa": np.asarray(sa_bv, f32),
        "bv_m": np.asarray(mha_in_b[2 * D:], f32),
        "bmo": np.asarray(mha_out_b, f32),
        "bf2": np.asarray(ffn_b2, f32),
        "ln1g": np.asarray(ln1_g, f32), "ln1b": np.asarray(ln1_b, f32),
        "ln2g": np.asarray(ln2_g, f32), "ln2b": np.asarray(ln2_b, f32),
        "ln3g": np.asarray(ln3_g, f32), "ln3b": np.asarray(ln3_b, f32),
    }

    kk = np.arange(S, dtype=np.int64)[:, None]
    in_maps = []
    for c in range(NC):
        b, s = c // 4, c % 4
        xT = np.ascontiguousarray(x_q[b].T.astype(bf))      # [D, S]
        qq = np.arange(T, dtype=np.int64)[None, :] + s * T
        m = np.where(kk <= qq, 1.0, 0.0).astype(bf)         # [S, T] 0/1 mult mask
        im = dict(shared)
        im["xqT_full"] = xT
        im["xqT_own"] = np.ascontiguousarray(xT[:, s * T:(s + 1) * T])
        im["x1T_own"] = np.ascontiguousarray(
            x1[b].T[:, s * T:(s + 1) * T].astype(bf))
        im["x2T_own"] = np.ascontiguousarray(
            x2[b].T[:, s * T:(s + 1) * T].astype(bf))
        im["xq_own_tm"] = np.ascontiguousarray(
            x_q[b, s * T:(s + 1) * T, :].astype(f32))
        im["maskT"] = m
        in_maps.append(im)
    return in_maps


def kernel(**inputs):
    global _NC_CACHE
    if _NC_CACHE is None:
        _NC_CACHE = build_nc()
    nc = _NC_CACHE
    in_maps = _prep_inputs(**{k: np.asarray(v) for k, v in inputs.items()})
    res = run_bass_kernel_spmd(nc, in_maps, core_ids=list(range(NC)))
    full = np.empty((B, S, D), np.float32)
    for c in range(NC):
        b, s = c // 4, c % 4
        full[b, s * T:(s + 1) * T, :] = res.results[c]["out"]
    return full
